# revision 3
# baseline (speedup 1.0000x reference)
"""InternLM3 custom attention on 8 TRN2 NeuronCores.

Sharding: heads 4-per-core for K/V projection + attention (qk_w/v_w
column-parallel by head); AllToAll converts the attention output from
head-sharded to sequence-sharded; o-projection runs sequence-parallel
(full o_w per core) so each core emits a [256, 2048] output slice.

All matmuls run as float32r (full-rate fp32 streaming mode, free dim
kept >= 256). Attention is computed transposed (S^T[k, q]) so softmax
probabilities feed the PV matmul directly as the moving operand with
no PE transposes; the softmax denominator rides along as a ones column
appended to V. Causality: strictly-upper k-chunks are skipped at block
granularity; diagonal-band blocks are zeroed post-exp with a sliding
slice of one [128, 896] host mask.
"""

import sys

sys.path.insert(0, "/opt/trn_rl_repo")

import numpy as np

import concourse.bass as bass
import concourse.tile as tile
from concourse import bacc, mybir
from concourse.bass import ds, ts
from concourse.bass_utils import run_bass_kernel_spmd

F32 = mybir.dt.float32
F32R = mybir.dt.float32r
NCORES = 8
S = 2048          # sequence
HID = 2048        # hidden
NH = 32           # total heads
HD = 64           # head dim
HPC = NH // NCORES      # heads per core = 4
DPC = HPC * HD          # head-dims per core = 256
SSL = S // NCORES       # output seq slice per core = 256
VW = 68                 # interleaved V stride: 64 dims + 1 ones + 3 pad
ROPE_THETA = 10000.0




def build_program(collective=True):
    nc = bacc.Bacc("TRN2", target_bir_lowering=False, debug=False,
                   num_devices=NCORES)

    # ---- I/O ----
    hidT = nc.dram_tensor("hidT", [HID, S], F32, kind="ExternalInput").ap()
    qkwT = nc.dram_tensor("qkwT", [HID, DPC], F32, kind="ExternalInput").ap()
    vwT = nc.dram_tensor("vwT", [HID, DPC], F32, kind="ExternalInput").ap()
    owT = nc.dram_tensor("owT", [HID, HID], F32, kind="ExternalInput").ap()
    xT_in = nc.dram_tensor("xT", [DPC, S], F32, kind="ExternalInput").ap()
    xTs_in = nc.dram_tensor("xTs", [DPC, S], F32, kind="ExternalInput").ap()
    cosT = nc.dram_tensor("cosT", [128, S], F32, kind="ExternalInput").ap()
    sinT = nc.dram_tensor("sinT", [128, S], F32, kind="ExternalInput").ap()
    maskT = nc.dram_tensor("maskT", [128, 896], F32, kind="ExternalInput").ap()
    out_sl = nc.dram_tensor("out_slice", [SSL, HID], F32,
                            kind="ExternalOutput").ap()

    with tile.TileContext(nc) as tc:
        with (
            nc.allow_low_precision(reason="float32r streaming mode, fp32 psum accum"),
            tc.tile_pool(name="const", bufs=1) as const,
            tc.tile_pool(name="dram", bufs=1, space="DRAM") as dram,
        ):
            # ---- persistent SBUF residents ----
            qkw_t = const.tile([128, 16, DPC], F32R)   # qk_w^T chunks
            nc.sync.dma_start(out=qkw_t[:],
                              in_=qkwT.rearrange("(n p) d -> p n d", p=128).bitcast(F32R))
            vw_t = const.tile([128, 16, DPC], F32R)
            nc.sync.dma_start(out=vw_t[:],
                              in_=vwT.rearrange("(n p) d -> p n d", p=128).bitcast(F32R))
            cos_t = const.tile([128, S], F32)
            nc.sync.dma_start(out=cos_t[:], in_=cosT)
            sin_t = const.tile([128, S], F32)
            nc.sync.dma_start(out=sin_t[:], in_=sinT)
            mask_t = const.tile([128, 896], F32)
            nc.sync.dma_start(out=mask_t[:], in_=maskT)
            ones_t = const.tile([1, 64], F32R)

            xt = const.tile([128, 2, S], F32R)         # X^T (2 head-pair tiles)
            nc.sync.dma_start(out=xt[:],
                              in_=xT_in.rearrange("(t p) s -> p t s", p=128).bitcast(F32R))
            kt = const.tile([128, 2, S], F32R)         # K^T, rope'd in place
            v_t = const.tile([128, 16, VW * HPC], F32R)  # V interleaved + ones
            att_t = const.tile([128, 2, S], F32)      # attn^T assembled
            # mask_t[:, 895] and row-0 cols>=384 are all 1.0 — reuse as ones
            nc.vector.tensor_copy(out=ones_t[:], in_=mask_t[0:1, 384:448])
            for h in range(HPC):
                for st in range(16):
                    nc.vector.tensor_copy(
                        out=v_t[:, st, VW * h + HD:VW * h + HD + 1],
                        in_=mask_t[:, 895:896])

            # =========== Phase A: K^T and V projections ===========
            with (
                tc.tile_pool(name="hq", bufs=6) as hpool,
                tc.tile_pool(name="psk", bufs=2, space="PSUM") as psk,
                tc.tile_pool(name="psv", bufs=4, space="PSUM") as psv,
            ):
                for sq in range(4):            # quarter of the sequence
                    pk = [psk.tile([128, 512], F32, tag='pk', name='pk') for _ in range(2)]
                    pv = [psv.tile([128, DPC], F32, tag='pv', name='pv') for _ in range(4)]
                    for hc in range(16):       # hidden-dim chunk
                        hq = hpool.tile([128, 512], F32R)
                        nc.sync.dma_start(
                            out=hq[:],
                            in_=hidT[ts(hc, 128), ts(sq, 512)].bitcast(F32R))
                        for m in range(2):
                            nc.tensor.matmul(
                                pk[m][:],
                                (qkw_t[:, hc, ts(m, 128)]),
                                (hq[:]),
                                start=(hc == 0), stop=(hc == 15))
                        for st4 in range(4):
                            nc.tensor.matmul(
                                pv[st4][:],
                                (hq[:, ts(st4, 128)]),
                                (vw_t[:, hc, :]),
                                start=(hc == 0), stop=(hc == 15))
                    for m in range(2):
                        nc.scalar.copy(out=kt[:, m, ts(sq, 512)], in_=pk[m][:])
                    for st4 in range(4):
                        for h in range(HPC):
                            nc.vector.tensor_copy(
                                out=v_t[:, sq * 4 + st4,
                                        ds(VW * h, HD)],
                                in_=pv[st4][:, ts(h, HD)])

            # =========== RoPE on X^T and K^T (in place) ===========
            with tc.tile_pool(name="sw", bufs=2) as swp:
                for t in range(2):
                    xs = swp.tile([128, S], F32, tag="sw")
                    nc.sync.dma_start(out=xs[:], in_=xTs_in[ts(t, 128), :])
                    nc.vector.tensor_mul(out=xt[:, t, :], in0=xt[:, t, :],
                                         in1=cos_t[:])
                    nc.vector.tensor_mul(out=xs[:], in0=xs[:], in1=sin_t[:])
                    nc.vector.tensor_add(out=xt[:, t, :], in0=xt[:, t, :],
                                         in1=xs[:])
                for t in range(2):
                    ks = swp.tile([128, S], F32, tag="sw")
                    # rotate_half row swap within each 64-row head block
                    for g in range(2):
                        b = 64 * g
                        nc.sync.dma_start(out=ks[b:b + 32, :],
                                          in_=kt[b + 32:b + 64, t, :].bitcast(F32))
                        nc.sync.dma_start(out=ks[b + 32:b + 64, :],
                                          in_=kt[b:b + 32, t, :].bitcast(F32))
                    nc.vector.tensor_mul(out=kt[:, t, :], in0=kt[:, t, :],
                                         in1=cos_t[:])
                    nc.vector.tensor_mul(out=ks[:], in0=ks[:], in1=sin_t[:])
                    nc.vector.tensor_add(out=kt[:, t, :], in0=kt[:, t, :],
                                         in1=ks[:])

            # =========== Phase B: attention per head ===========
            with (
                tc.tile_pool(name="pp", bufs=6) as ppool,
                tc.tile_pool(name="pss", bufs=3, space="PSUM") as pss,
                tc.tile_pool(name="pspv", bufs=2, space="PSUM") as pspv,
                tc.tile_pool(name="psbc", bufs=2, space="PSUM") as psbc,
                tc.tile_pool(name="rr", bufs=4) as rrp,
            ):
                for h in range(HPC):
                    hp = 64 * (h % 2)       # partition offset of this head
                    htl = h // 2            # which head-pair tile
                    for j in range(4):      # q block of 512
                        q0 = 512 * j
                        pvp = pspv.tile([HD + 1, 512], F32, tag='pvp')
                        nk = 4 * (j + 1)    # causal: k chunks 0..nk-1
                        for i in range(nk):
                            k0 = 128 * i
                            sp = pss.tile([128, 512], F32, tag='sp')
                            nc.tensor.matmul(
                                sp[:],
                                (kt[hp:hp + HD, htl, ts(i, 128)]),
                                (xt[hp:hp + HD, htl, ds(q0, 512)]),
                                start=True, stop=True)
                            pt = ppool.tile([128, 512], F32R, tag="pt")
                            nc.scalar.activation(
                                out=pt[:], in_=sp[:],
                                func=mybir.ActivationFunctionType.Exp,
                                scale=0.125)
                            r = k0 - q0
                            if r >= 0:      # diagonal band: causal mask
                                nc.vector.tensor_mul(
                                    out=pt[:], in0=pt[:],
                                    in1=mask_t[:, ds(384 - r, 512)])
                            nc.tensor.matmul(
                                pvp[:],
                                (v_t[:, i, ds(VW * h, HD + 1)]),
                                (pt[:]),
                                start=(i == 0), stop=(i == nk - 1))
                        # divide by denominator (row HD) & place into att_t
                        rec = rrp.tile([1, 512], F32R, tag="rec")
                        nc.vector.reciprocal(out=rec[:], in_=pvp[HD:HD + 1, :])
                        bc = psbc.tile([64, 512], F32, tag='bc')
                        nc.tensor.matmul(bc[:], (ones_t[:]),
                                         (rec[:]), start=True, stop=True)
                        nc.scalar.copy(out=att_t[hp:hp + HD, htl, ds(q0, 512)],
                                       in_=pvp[0:HD, :])
                        nc.vector.tensor_mul(
                            out=att_t[hp:hp + HD, htl, ds(q0, 512)],
                            in0=att_t[hp:hp + HD, htl, ds(q0, 512)],
                            in1=bc[:])

            # =========== Phase C: AllToAll + o-projection ===========
            a2a_in = dram.tile([NCORES, DPC, SSL], F32)
            a2a_out = dram.tile([S, SSL], F32)
            for t in range(2):
                for d in range(NCORES):
                    nc.sync.dma_start(out=a2a_in[d, ts(t, 128), :],
                                      in_=att_t[:, t, ts(d, SSL)])
            if collective:
                nc.gpsimd.collective_compute(
                    "AllToAll",
                    mybir.AluOpType.bypass,
                    replica_groups=[list(range(NCORES))],
                    ins=[a2a_in[:].opt()],
                    outs=[a2a_out[:].opt()],
                )
            else:
                # timeline-sim mock: same-size DRAM->DRAM move
                nc.sync.dma_start(
                    out=a2a_out[:],
                    in_=a2a_in[:].rearrange("d p s -> (d p) s"))

            with (
                tc.tile_pool(name="af", bufs=1) as afp,
                tc.tile_pool(name="ow", bufs=4) as owp,
                tc.tile_pool(name="ob", bufs=1) as obp,
                tc.tile_pool(name="pso", bufs=8, space="PSUM") as pso,
            ):
                afull = afp.tile([128, 16, SSL], F32R)
                nc.sync.dma_start(
                    out=afull[:],
                    in_=a2a_out[:].rearrange("(n p) s -> p n s", p=128).bitcast(F32R))
                osb = obp.tile([128, 2, HID], F32)
                po = [[pso.tile([128, 512], F32, tag='po', name='po') for t in range(2)]
                      for ob in range(4)]
                for hc in range(16):
                    ow_t = owp.tile([128, HID], F32R, tag="ow")
                    nc.sync.dma_start(out=ow_t[:], in_=owT[ts(hc, 128), :].bitcast(F32R))
                    for ob in range(4):
                        for t in range(2):
                            nc.tensor.matmul(
                                po[ob][t][:],
                                (afull[:, hc, ts(t, 128)]),
                                (ow_t[:, ts(ob, 512)]),
                                start=(hc == 0), stop=(hc == 15))
                for ob in range(4):
                    for t in range(2):
                        nc.scalar.copy(out=osb[:, t, ts(ob, 512)],
                                       in_=po[ob][t][:])
                for t in range(2):
                    nc.sync.dma_start(out=out_sl[ts(t, 128), :],
                                      in_=osb[:, t, :])

    nc.compile()
    return nc


_PROGRAM = None


def _host_inputs(hidden_states, qk_w, v_w, o_w, position_ids):
    hs = np.asarray(hidden_states, dtype=np.float32)[0]          # [S, HID]
    qk_w = np.asarray(qk_w, dtype=np.float32)
    v_w = np.asarray(v_w, dtype=np.float32)
    o_w = np.asarray(o_w, dtype=np.float32)
    pos = np.asarray(position_ids)[0].astype(np.float64)         # [S]

    hidT = np.ascontiguousarray(hs.T)                            # [HID, S]
    owT = np.ascontiguousarray(o_w.T)                            # [HID, HID]

    inv_freq = 1.0 / (ROPE_THETA ** (np.arange(0, HD, 2, dtype=np.float64) / HD))
    freqs = pos[None, :] * inv_freq[:, None]                     # [32, S]
    emb = np.concatenate([freqs, freqs], axis=0)                 # [64, S]
    cos1 = np.cos(emb).astype(np.float32)
    sin1 = np.sin(emb).astype(np.float32)
    sin_signed = sin1.copy()
    sin_signed[:HD // 2] *= -1.0                                 # fold rotate sign
    cosT = np.tile(cos1, (2, 1)).astype(np.float32)              # [128, S]
    sinT = np.tile(sin_signed, (2, 1)).astype(np.float32)

    kl = np.arange(128)[:, None]
    u = np.arange(896)[None, :]
    maskT = (u >= kl + 384).astype(np.float32)                   # [128, 896]

    in_maps = []
    for c in range(NCORES):
        rows = slice(DPC * c, DPC * (c + 1))
        xT = hidT[rows]                                          # [256, S]
        xTs = np.empty_like(xT)                                  # rotate_half rows
        for h in range(HPC):
            b = HD * h
            xTs[b:b + 32] = xT[b + 32:b + 64]
            xTs[b + 32:b + 64] = xT[b:b + 32]
        in_maps.append({
            "hidT": hidT,
            "qkwT": np.ascontiguousarray(qk_w[rows].T),          # [HID, 256]
            "vwT": np.ascontiguousarray(v_w[rows].T),
            "owT": owT,
            "xT": np.ascontiguousarray(xT),
            "xTs": np.ascontiguousarray(xTs),
            "cosT": cosT,
            "sinT": sinT,
            "maskT": maskT,
        })
    return in_maps


def kernel(hidden_states, qk_w, v_w, o_w, position_ids, **extra):
    global _PROGRAM
    if _PROGRAM is None:
        _PROGRAM = build_program()
    in_maps = _host_inputs(hidden_states, qk_w, v_w, o_w, position_ids)
    res = run_bass_kernel_spmd(_PROGRAM, in_maps, list(range(NCORES)))
    out = np.concatenate([res.results[c]["out_slice"]
                          for c in range(NCORES)], axis=0)
    return out.reshape(1, S, HID).astype(np.float32)



# revision 25
# speedup vs baseline: 1.0849x; 1.0849x over previous
"""InternLM3 custom attention on 8 TRN2 NeuronCores.

Sharding: heads 4-per-core for K/V projection + attention (qk_w/v_w
column-parallel by head); AllToAll converts the attention output from
head-sharded to sequence-sharded; o-projection runs sequence-parallel
(full o_w per core) so each core emits a [256, 2048] output slice.

v2: bf16 streaming path (fp32 PSUM accumulation), software-pipelined
projection -> RoPE -> attention per 512-sequence chunk so the exp
(Activation engine) overlaps the projection matmuls (PE), big resident
SBUF tiles loaded with few large DMAs split across both HWDGE queues
(SP + Activation), o_w prefetched during attention. X-RoPE is folded
into host prep. Attention is computed transposed (S^T[k, q]) so softmax
probabilities feed the PV matmul directly; the softmax denominator
rides along as a ones column appended to V, and its broadcast
reciprocal shares the attention PSUM bank (partitions 64..127).
Causality: strictly-upper k-blocks skipped; diagonal blocks compute
exp/PV only on columns >= r with one [128,128] triangular mask.
"""

import sys

sys.path.insert(0, "/opt/trn_rl_repo")

import numpy as np
import ml_dtypes

import concourse.bass as bass
import concourse.tile as tile
from concourse import bacc, mybir
from concourse.bass import ds, ts
from concourse.bass_utils import run_bass_kernel_spmd

F32 = mybir.dt.float32
BF16 = mybir.dt.bfloat16
NCORES = 8
S = 2048          # sequence
HID = 2048        # hidden
NH = 32           # total heads
HD = 64           # head dim
HPC = NH // NCORES      # heads per core = 4
DPC = HPC * HD          # head-dims per core = 256
SSL = S // NCORES       # output seq slice per core = 256
VW = 66                 # interleaved V stride: 64 dims + 1 ones + 1 pad
ROPE_THETA = 10000.0


def build_program(collective=True, dbg=False):
    nc = bacc.Bacc("TRN2", target_bir_lowering=False, debug=False,
                   num_devices=NCORES)

    # ---- I/O (bf16 streaming; fp32 out) ----
    hidT = nc.dram_tensor("hidT", [HID, S], BF16, kind="ExternalInput").ap()
    qkwT = nc.dram_tensor("qkwT", [HID, DPC], BF16, kind="ExternalInput").ap()
    vwT = nc.dram_tensor("vwT", [HID, DPC], BF16, kind="ExternalInput").ap()
    owT = nc.dram_tensor("owT", [HID, HID], BF16, kind="ExternalInput").ap()
    xT_in = nc.dram_tensor("xT", [DPC, S], BF16, kind="ExternalInput").ap()
    cosT = nc.dram_tensor("cosT", [128, S], BF16, kind="ExternalInput").ap()
    sinT = nc.dram_tensor("sinT", [128, S], BF16, kind="ExternalInput").ap()
    triT = nc.dram_tensor("triT", [128, 128], BF16, kind="ExternalInput").ap()
    permT = nc.dram_tensor("permT", [128, 128], BF16, kind="ExternalInput").ap()
    out_sl = nc.dram_tensor("out_slice", [SSL, HID], F32,
                            kind="ExternalOutput").ap()
    if dbg:
        kt_out = nc.dram_tensor("kt_out", [128, 2 * S], BF16,
                                kind="ExternalOutput").ap()
        vt_out = nc.dram_tensor("vt_out", [128, 16 * VW * HPC], BF16,
                                kind="ExternalOutput").ap()
        att_out = nc.dram_tensor("att_out", [128, 2 * S], BF16,
                                 kind="ExternalOutput").ap()
        afu_out = nc.dram_tensor("afu_out", [128, 16 * SSL], BF16,
                                 kind="ExternalOutput").ap()

    with tile.TileContext(nc) as tc:
        with (
            nc.allow_low_precision(reason="bf16 streaming, fp32 psum accum"),
            tc.tile_pool(name="const", bufs=1) as const,
            tc.tile_pool(name="dram", bufs=1, space="DRAM") as dram,
        ):
            # ---- persistent SBUF residents ----
            qkw_t = const.tile([128, 16, DPC], BF16)
            vw_t = const.tile([128, 16, DPC], BF16)
            xt = const.tile([128, 2, S], BF16)      # X^T, rope'd on host
            cos_t = const.tile([128, S], BF16)
            sin_t = const.tile([128, S], BF16)
            tri_t = const.tile([128, 128], BF16)
            perm_t = const.tile([128, 128], BF16)
            ow_t = const.tile([128, 16, HID], BF16)
            kt = const.tile([128, 2, S], BF16)      # K^T, rope'd in place
            v_t = const.tile([128, 16, VW * HPC], BF16)
            att_t = const.tile([128, 2, S], BF16)   # attn^T assembled
            ones_t = const.tile([1, HD], BF16)

            nc.vector.memset(ones_t[:], 1.0)
            # ones column of V (denominator accumulator)
            nc.vector.memset(
                v_t[:].rearrange("p st (h w) -> p st h w", w=VW)[:, :, :, HD:HD + 1],
                1.0)

            # ===== pipelined: per 512-seq chunk, project K/V, rope K,
            # ===== then attention q-block j=sq (needs K/V chunks <= sq).
            with (
                tc.tile_pool(name="hidp", bufs=1) as hidp,
                tc.tile_pool(name="psk", bufs=2, space="PSUM") as psk,
                tc.tile_pool(name="psv", bufs=2, space="PSUM") as psv,
                tc.tile_pool(name="pss", bufs=2, space="PSUM") as pss,
                tc.tile_pool(name="pspv", bufs=2, space="PSUM") as pspv,
                tc.tile_pool(name="sw", bufs=2) as swp,
                tc.tile_pool(name="pp", bufs=4) as ppool,
                tc.tile_pool(name="rr", bufs=2) as rrp,
            ):
                hid_t = hidp.tile([128, 16, S], BF16)
                # SP queue: split first loads so K matmuls start early;
                # vw only needed once the K half of chunk 0 is done.
                hidr = hidT.rearrange("(n p) s -> p n s", p=128)
                nc.sync.dma_start(out=qkw_t[:, 0:8, :],
                                  in_=qkwT.rearrange("(n p) d -> p n d",
                                                     p=128)[:, 0:8, :])
                nc.sync.dma_start(out=hid_t[:, 0:8, ts(0, 512)],
                                  in_=hidr[:, 0:8, ts(0, 512)])
                nc.sync.dma_start(out=qkw_t[:, 8:16, :],
                                  in_=qkwT.rearrange("(n p) d -> p n d",
                                                     p=128)[:, 8:16, :])
                nc.sync.dma_start(out=hid_t[:, 8:16, ts(0, 512)],
                                  in_=hidr[:, 8:16, ts(0, 512)])
                nc.sync.dma_start(out=vw_t[:],
                                  in_=vwT.rearrange("(n p) d -> p n d", p=128))
                for sq in range(1, 4):
                    nc.sync.dma_start(out=hid_t[:, :, ts(sq, 512)],
                                      in_=hidr[:, :, ts(sq, 512)])
                # o_w prefetch: after the phase A loads so it doesn't
                # steal DMA bandwidth from them; lands well before o-proj.
                nc.sync.dma_start(out=ow_t[:],
                                  in_=owT.rearrange("(n p) d -> p n d", p=128))
                # ACT queue: rope/attention consts (small, needed early).
                nc.scalar.dma_start(out=cos_t[:], in_=cosT)
                nc.scalar.dma_start(out=sin_t[:], in_=sinT)
                nc.scalar.dma_start(
                    out=xt[:], in_=xT_in.rearrange("(t p) s -> p t s", p=128))
                nc.scalar.dma_start(out=tri_t[:], in_=triT)
                nc.scalar.dma_start(out=perm_t[:], in_=permT)
                for sq in range(4):
                    sqs = ds(512 * sq, 512)
                    # ---- phase A chunk: K^T then V for seq block sq ----
                    pk = [psk.tile([128, 512], F32, tag='pk', name='pk')
                          for _ in range(2)]
                    for hc in range(16):
                        for m in range(2):
                            nc.tensor.matmul(
                                pk[m][:],
                                (qkw_t[:, hc, ts(m, 128)]),
                                (hid_t[:, hc, sqs]),
                                start=(hc == 0), stop=(hc == 15))
                    # K: copy to bf16, rotate-half via PE permutation matmul,
                    # rope in place (sin sign folded on host)
                    ks = swp.tile([128, 2, 512], BF16, tag="sw")
                    for t in range(2):
                        nc.vector.tensor_copy(out=kt[:, t, sqs], in_=pk[t][:])
                    for t in range(2):
                        ksp = pss.tile([128, 512], F32, tag='sp')
                        nc.tensor.matmul(ksp[:], (perm_t[:]),
                                         (kt[:, t, sqs]),
                                         start=True, stop=True)
                        nc.vector.tensor_mul(out=ks[:, t, :], in0=ksp[:],
                                             in1=sin_t[:, sqs])
                        nc.vector.tensor_mul(out=kt[:, t, sqs],
                                             in0=kt[:, t, sqs],
                                             in1=cos_t[:, sqs])
                        nc.vector.tensor_add(out=kt[:, t, sqs],
                                             in0=kt[:, t, sqs],
                                             in1=ks[:, t, :])
                    # V: one psum group per bank slot (start=True zeroes the
                    # whole 2KB zero-region, so groups must not share a bank)
                    for st4 in range(4):
                        pvt = psv.tile([128, 256], F32, tag='pv', name='pv')
                        for hc in range(16):
                            nc.tensor.matmul(
                                pvt[:],
                                (hid_t[:, hc, ds(512 * sq + 128 * st4, 128)]),
                                (vw_t[:, hc, :]),
                                start=(hc == 0), stop=(hc == 15))
                        nc.vector.tensor_copy(
                            out=v_t[:, sq * 4 + st4, :].rearrange(
                                "p (h w) -> p h w", w=VW)[:, :, 0:HD],
                            in_=pvt[:].rearrange("p (h d) -> p h d", d=HD))

                    # ---- phase B: attention q-block j == sq, all 4 heads ----
                    j = sq
                    q0 = 512 * j
                    nk = 4 * (j + 1)
                    for h in range(HPC):
                        hp = HD * (h % 2)
                        htl = h // 2
                        pvp = pspv.tile([HD + 1, 512], F32, tag='pvp')
                        for i in range(nk):
                            r = 128 * i - q0
                            w0 = max(r, 0)
                            sp = pss.tile([128, 512], F32, tag='sp')
                            nc.tensor.matmul(
                                sp[:, ds(w0, 512 - w0)],
                                (kt[hp:hp + HD, htl, ts(i, 128)]),
                                (xt[hp:hp + HD, htl, ds(q0 + w0, 512 - w0)]),
                                start=True, stop=True)
                            pt = ppool.tile([128, 512], BF16, tag="pt")
                            nc.scalar.activation(
                                out=pt[:, ds(w0, 512 - w0)],
                                in_=sp[:, ds(w0, 512 - w0)],
                                func=mybir.ActivationFunctionType.Exp,
                                scale=0.125)
                            if r >= 0:   # diagonal: ragged triangle mask
                                nc.vector.tensor_mul(
                                    out=pt[:, ds(r, 128)], in0=pt[:, ds(r, 128)],
                                    in1=tri_t[:])
                            nc.tensor.matmul(
                                pvp[0:HD + 1, ds(w0, 512 - w0)],
                                (v_t[:, i, ds(VW * h, HD + 1)]),
                                (pt[:, ds(w0, 512 - w0)]),
                                start=(i == 0), stop=(i == nk - 1))
                        # denominator: reciprocal row, broadcast into the
                        # unused partitions 64..127 of the same psum bank
                        rec = rrp.tile([1, 512], BF16, tag="rec")
                        nc.vector.reciprocal(out=rec[:], in_=pvp[HD:HD + 1, :])
                        bc = pss.tile([HD, 512], F32, tag='sp')
                        nc.tensor.matmul(bc[:], (ones_t[:]),
                                         (rec[:]), start=True, stop=True)
                        bcs = rrp.tile([HD, 512], BF16, tag="bcs")
                        nc.vector.tensor_copy(out=bcs[:], in_=bc[:])
                        nc.vector.tensor_mul(
                            out=att_t[hp:hp + HD, htl, ds(q0, 512)],
                            in0=pvp[0:HD, :],
                            in1=bcs[:])

            if dbg:
                nc.sync.dma_start(out=kt_out[:],
                                  in_=kt[:].rearrange("p t s -> p (t s)"))
                nc.sync.dma_start(out=vt_out[:],
                                  in_=v_t[:].rearrange("p a b -> p (a b)"))
                nc.sync.dma_start(out=att_out[:],
                                  in_=att_t[:].rearrange("p t s -> p (t s)"))

            # =========== AllToAll: head-sharded -> seq-sharded ===========
            a2a_in = dram.tile([NCORES, DPC, SSL], BF16)
            a2a_out = dram.tile([S, SSL], BF16)
            for t in range(2):
                nc.sync.dma_start(
                    out=a2a_in[:, ts(t, 128), :].rearrange("d p s -> p d s"),
                    in_=att_t[:, t, :].rearrange("p (d s) -> p d s", d=NCORES))
            if collective:
                nc.gpsimd.collective_compute(
                    "AllToAll",
                    mybir.AluOpType.bypass,
                    replica_groups=[list(range(NCORES))],
                    ins=[a2a_in[:].opt()],
                    outs=[a2a_out[:].opt()],
                )
            else:
                # timeline-sim mock: same-size DRAM->DRAM move
                nc.sync.dma_start(
                    out=a2a_out[:],
                    in_=a2a_in[:].rearrange("d p s -> (d p) s"))

            # =========== o-projection (sequence-parallel) ===========
            with (
                tc.tile_pool(name="af", bufs=1) as afp,
                tc.tile_pool(name="ob", bufs=1) as obp,
                tc.tile_pool(name="pso", bufs=8, space="PSUM") as pso,
            ):
                afull = afp.tile([128, 16, SSL], BF16)
                nc.sync.dma_start(
                    out=afull[:],
                    in_=a2a_out[:].rearrange("(n p) s -> p n s", p=128))
                if dbg:
                    nc.sync.dma_start(out=afu_out[:],
                                      in_=afull[:].rearrange("p a b -> p (a b)"))
                osb = obp.tile([128, 2, HID], F32)
                po = [[pso.tile([128, 512], F32, tag='po', name='po')
                       for t in range(2)] for ob in range(4)]
                for hc in range(16):
                    for ob in range(4):
                        for t in range(2):
                            nc.tensor.matmul(
                                po[ob][t][:],
                                (afull[:, hc, ts(t, 128)]),
                                (ow_t[:, hc, ts(ob, 512)]),
                                start=(hc == 0), stop=(hc == 15))
                for t in range(2):
                    for ob in range(4):
                        nc.scalar.copy(out=osb[:, t, ts(ob, 512)],
                                       in_=po[ob][t][:])
                    nc.sync.dma_start(out=out_sl[ts(t, 128), :],
                                      in_=osb[:, t, :])

    nc.compile()
    return nc


_PROGRAM = None


def _host_inputs(hidden_states, qk_w, v_w, o_w, position_ids):
    bf16 = ml_dtypes.bfloat16
    hs = np.asarray(hidden_states, dtype=np.float32)[0]          # [S, HID]
    qk_w = np.asarray(qk_w, dtype=np.float32)
    v_w = np.asarray(v_w, dtype=np.float32)
    o_w = np.asarray(o_w, dtype=np.float32)
    pos = np.asarray(position_ids)[0].astype(np.float64)         # [S]

    hidT = np.ascontiguousarray(hs.T)                            # [HID, S]
    hidT_bf = hidT.astype(bf16)
    owT_bf = np.ascontiguousarray(o_w.T).astype(bf16)            # [HID, HID]

    inv_freq = 1.0 / (ROPE_THETA ** (np.arange(0, HD, 2, dtype=np.float64) / HD))
    freqs = pos[None, :] * inv_freq[:, None]                     # [32, S]
    emb = np.concatenate([freqs, freqs], axis=0)                 # [64, S]
    cos1 = np.cos(emb).astype(np.float32)
    sin1 = np.sin(emb).astype(np.float32)
    sin_signed = sin1.copy()
    sin_signed[:HD // 2] *= -1.0                                 # fold rotate sign
    cosT = np.tile(cos1, (2, 1)).astype(bf16)                    # [128, S]
    sinT = np.tile(sin_signed, (2, 1)).astype(bf16)

    kl = np.arange(128)[:, None]
    u = np.arange(128)[None, :]
    triT = (u >= kl).astype(bf16)                                # [128, 128]

    # rotate-half row permutation (symmetric involution, per 64-row head)
    idx = np.arange(128)
    src = (idx // HD) * HD + (idx % HD + HD // 2) % HD
    permT = np.zeros((128, 128), np.float32)
    permT[idx, src] = 1.0
    permT = permT.astype(bf16)

    in_maps = []
    for c in range(NCORES):
        rows = slice(DPC * c, DPC * (c + 1))
        xT = hidT[rows]                                          # [256, S] fp32
        # host-side X RoPE: x*cos + rotate_half(x)*sin per 64-row head
        xTs = np.empty_like(xT)
        for h in range(HPC):
            b = HD * h
            xTs[b:b + 32] = -xT[b + 32:b + HD]
            xTs[b + 32:b + HD] = xT[b:b + 32]
        cs = np.tile(cos1, (HPC, 1))                             # [256, S]
        sn = np.tile(sin1, (HPC, 1))
        xTr = (xT * cs + xTs * sn).astype(bf16)
        in_maps.append({
            "hidT": hidT_bf,
            "qkwT": np.ascontiguousarray(qk_w[rows].T).astype(bf16),
            "vwT": np.ascontiguousarray(v_w[rows].T).astype(bf16),
            "owT": owT_bf,
            "xT": xTr,
            "cosT": cosT,
            "sinT": sinT,
            "triT": triT,
            "permT": permT,
        })
    return in_maps


def kernel(hidden_states, qk_w, v_w, o_w, position_ids, **extra):
    global _PROGRAM
    if _PROGRAM is None:
        _PROGRAM = build_program()
    in_maps = _host_inputs(hidden_states, qk_w, v_w, o_w, position_ids)
    res = run_bass_kernel_spmd(_PROGRAM, in_maps, list(range(NCORES)))
    out = np.concatenate([res.results[c]["out_slice"]
                          for c in range(NCORES)], axis=0)
    return out.reshape(1, S, HID).astype(np.float32)


# revision 29
# speedup vs baseline: 1.1318x; 1.0433x over previous
"""InternLM3 custom attention on 8 TRN2 NeuronCores.

Sharding: heads 4-per-core for K/V projection + attention (qk_w/v_w
column-parallel by head); AllToAll converts the attention output from
head-sharded to sequence-sharded; o-projection runs sequence-parallel
(full o_w per core) so each core emits a [256, 2048] output slice.

v2: bf16 streaming path (fp32 PSUM accumulation), software-pipelined
projection -> RoPE -> attention per 512-sequence chunk so the exp
(Activation engine) overlaps the projection matmuls (PE), big resident
SBUF tiles loaded with few large DMAs split across both HWDGE queues
(SP + Activation), o_w prefetched during attention. X-RoPE is folded
into host prep. Attention is computed transposed (S^T[k, q]) so softmax
probabilities feed the PV matmul directly; the softmax denominator
rides along as a ones column appended to V, and its broadcast
reciprocal shares the attention PSUM bank (partitions 64..127).
Causality: strictly-upper k-blocks skipped; diagonal blocks compute
exp/PV only on columns >= r with one [128,128] triangular mask.
"""

import sys

sys.path.insert(0, "/opt/trn_rl_repo")

import numpy as np
import ml_dtypes

import concourse.bass as bass
import concourse.tile as tile
from concourse import bacc, mybir
from concourse.bass import ds, ts
from concourse.bass_utils import run_bass_kernel_spmd

F32 = mybir.dt.float32
BF16 = mybir.dt.bfloat16
NCORES = 8
S = 2048          # sequence
HID = 2048        # hidden
NH = 32           # total heads
HD = 64           # head dim
HPC = NH // NCORES      # heads per core = 4
DPC = HPC * HD          # head-dims per core = 256
SSL = S // NCORES       # output seq slice per core = 256
VW = 66                 # interleaved V stride: 64 dims + 1 ones + 1 pad
ROPE_THETA = 10000.0

# packed input blob layout (bf16 elements, per core)
_BLOB_SPEC = [
    ("hidT", HID * S),
    ("qkwT", HID * DPC),
    ("vwT", HID * DPC),
    ("owT", HID * HID),
    ("xT", DPC * S),
    ("cosT", 128 * S),
    ("sinT", 128 * S),
    ("triT", 128 * 128),
    ("permT", 128 * 128),
]
BLOB_OFFS = {}
_off = 0
for _nm, _n in _BLOB_SPEC:
    BLOB_OFFS[_nm] = (_off, _n)
    _off += _n
BLOB_ELEMS = _off


def build_program(collective=True, dbg=False):
    nc = bacc.Bacc("TRN2", target_bir_lowering=False, debug=False,
                   num_devices=NCORES)

    # ---- I/O: one packed bf16 input blob (fewer per-iteration buffer
    # ---- handles on the dispatch path); fp32 out ----
    blob = nc.dram_tensor("blob", [BLOB_ELEMS], BF16, kind="ExternalInput").ap()

    def bslice(name):
        off, n = BLOB_OFFS[name]
        return blob[ds(off, n)]

    hidT = bslice("hidT").rearrange("(n p s) -> p n s", p=128, s=S)
    qkwT = bslice("qkwT").rearrange("(n p d) -> p n d", p=128, d=DPC)
    vwT = bslice("vwT").rearrange("(n p d) -> p n d", p=128, d=DPC)
    owT = bslice("owT").rearrange("(n p d) -> p n d", p=128, d=HID)
    xT_in = bslice("xT").rearrange("(t p s) -> p t s", p=128, s=S)
    cosT = bslice("cosT").rearrange("(p s) -> p s", p=128)
    sinT = bslice("sinT").rearrange("(p s) -> p s", p=128)
    triT = bslice("triT").rearrange("(p q) -> p q", p=128)
    permT = bslice("permT").rearrange("(p q) -> p q", p=128)
    out_sl = nc.dram_tensor("out_slice", [SSL, HID], F32,
                            kind="ExternalOutput").ap()
    if dbg:
        kt_out = nc.dram_tensor("kt_out", [128, 2 * S], BF16,
                                kind="ExternalOutput").ap()
        vt_out = nc.dram_tensor("vt_out", [128, 16 * VW * HPC], BF16,
                                kind="ExternalOutput").ap()
        att_out = nc.dram_tensor("att_out", [128, 2 * S], BF16,
                                 kind="ExternalOutput").ap()
        afu_out = nc.dram_tensor("afu_out", [128, 16 * SSL], BF16,
                                 kind="ExternalOutput").ap()

    with tile.TileContext(nc) as tc:
        with (
            nc.allow_low_precision(reason="bf16 streaming, fp32 psum accum"),
            tc.tile_pool(name="const", bufs=1) as const,
            tc.tile_pool(name="dram", bufs=1, space="DRAM") as dram,
        ):
            # ---- persistent SBUF residents ----
            qkw_t = const.tile([128, 16, DPC], BF16)
            vw_t = const.tile([128, 16, DPC], BF16)
            xt = const.tile([128, 2, S], BF16)      # X^T, rope'd on host
            cos_t = const.tile([128, S], BF16)
            sin_t = const.tile([128, S], BF16)
            tri_t = const.tile([128, 128], BF16)
            perm_t = const.tile([128, 128], BF16)
            ow_t = const.tile([128, 16, HID], BF16)
            kt = const.tile([128, 2, S], BF16)      # K^T, rope'd in place
            v_t = const.tile([128, 16, VW * HPC], BF16)
            att_t = const.tile([128, 2, S], BF16)   # attn^T assembled
            ones_t = const.tile([1, HD], BF16)

            nc.vector.memset(ones_t[:], 1.0)
            # ones column of V (denominator accumulator)
            nc.vector.memset(
                v_t[:].rearrange("p st (h w) -> p st h w", w=VW)[:, :, :, HD:HD + 1],
                1.0)

            # ===== pipelined: per 512-seq chunk, project K/V, rope K,
            # ===== then attention q-block j=sq (needs K/V chunks <= sq).
            with (
                tc.tile_pool(name="hidp", bufs=1) as hidp,
                tc.tile_pool(name="psk", bufs=2, space="PSUM") as psk,
                tc.tile_pool(name="psv", bufs=2, space="PSUM") as psv,
                tc.tile_pool(name="pss", bufs=2, space="PSUM") as pss,
                tc.tile_pool(name="pspv", bufs=2, space="PSUM") as pspv,
                tc.tile_pool(name="sw", bufs=2) as swp,
                tc.tile_pool(name="pp", bufs=4) as ppool,
                tc.tile_pool(name="rr", bufs=2) as rrp,
            ):
                hid_t = hidp.tile([128, 16, S], BF16)
                # SP queue: split first loads so K matmuls start early;
                # vw only needed once the K half of chunk 0 is done.
                nc.sync.dma_start(out=qkw_t[:, 0:8, :], in_=qkwT[:, 0:8, :])
                nc.sync.dma_start(out=hid_t[:, 0:8, ts(0, 512)],
                                  in_=hidT[:, 0:8, ts(0, 512)])
                nc.sync.dma_start(out=qkw_t[:, 8:16, :], in_=qkwT[:, 8:16, :])
                nc.sync.dma_start(out=hid_t[:, 8:16, ts(0, 512)],
                                  in_=hidT[:, 8:16, ts(0, 512)])
                nc.sync.dma_start(out=vw_t[:], in_=vwT)
                for sq in range(1, 4):
                    nc.sync.dma_start(out=hid_t[:, :, ts(sq, 512)],
                                      in_=hidT[:, :, ts(sq, 512)])
                # o_w prefetch: after the phase A loads so it doesn't
                # steal DMA bandwidth from them; lands well before o-proj.
                nc.sync.dma_start(out=ow_t[:], in_=owT)
                # ACT queue: rope/attention consts (small, needed early).
                nc.scalar.dma_start(out=cos_t[:], in_=cosT)
                nc.scalar.dma_start(out=sin_t[:], in_=sinT)
                nc.scalar.dma_start(out=xt[:], in_=xT_in)
                nc.scalar.dma_start(out=tri_t[:], in_=triT)
                nc.scalar.dma_start(out=perm_t[:], in_=permT)
                for sq in range(4):
                    sqs = ds(512 * sq, 512)
                    # ---- phase A chunk: K^T then V for seq block sq ----
                    pk = [psk.tile([128, 512], F32, tag='pk', name='pk')
                          for _ in range(2)]
                    for hc in range(16):
                        for m in range(2):
                            nc.tensor.matmul(
                                pk[m][:],
                                (qkw_t[:, hc, ts(m, 128)]),
                                (hid_t[:, hc, sqs]),
                                start=(hc == 0), stop=(hc == 15))
                    # K: copy to bf16, rotate-half via PE permutation matmul,
                    # rope in place (sin sign folded on host)
                    ks = swp.tile([128, 2, 512], BF16, tag="sw")
                    for t in range(2):
                        nc.vector.tensor_copy(out=kt[:, t, sqs], in_=pk[t][:])
                    for t in range(2):
                        ksp = pss.tile([128, 512], F32, tag='sp')
                        nc.tensor.matmul(ksp[:], (perm_t[:]),
                                         (kt[:, t, sqs]),
                                         start=True, stop=True)
                        nc.vector.tensor_mul(out=ks[:, t, :], in0=ksp[:],
                                             in1=sin_t[:, sqs])
                        nc.vector.tensor_mul(out=kt[:, t, sqs],
                                             in0=kt[:, t, sqs],
                                             in1=cos_t[:, sqs])
                        nc.vector.tensor_add(out=kt[:, t, sqs],
                                             in0=kt[:, t, sqs],
                                             in1=ks[:, t, :])
                    # V: one psum group per bank slot (start=True zeroes the
                    # whole 2KB zero-region, so groups must not share a bank)
                    for st4 in range(4):
                        pvt = psv.tile([128, 256], F32, tag='pv', name='pv')
                        for hc in range(16):
                            nc.tensor.matmul(
                                pvt[:],
                                (hid_t[:, hc, ds(512 * sq + 128 * st4, 128)]),
                                (vw_t[:, hc, :]),
                                start=(hc == 0), stop=(hc == 15))
                        nc.vector.tensor_copy(
                            out=v_t[:, sq * 4 + st4, :].rearrange(
                                "p (h w) -> p h w", w=VW)[:, :, 0:HD],
                            in_=pvt[:].rearrange("p (h d) -> p h d", d=HD))

                    # ---- phase B: attention q-block j == sq, all 4 heads ----
                    j = sq
                    q0 = 512 * j
                    nk = 4 * (j + 1)
                    for h in range(HPC):
                        hp = HD * (h % 2)
                        htl = h // 2
                        pvp = pspv.tile([HD + 1, 512], F32, tag='pvp')
                        for i in range(nk):
                            r = 128 * i - q0
                            w0 = max(r, 0)
                            sp = pss.tile([128, 512], F32, tag='sp')
                            nc.tensor.matmul(
                                sp[:, ds(w0, 512 - w0)],
                                (kt[hp:hp + HD, htl, ts(i, 128)]),
                                (xt[hp:hp + HD, htl, ds(q0 + w0, 512 - w0)]),
                                start=True, stop=True)
                            pt = ppool.tile([128, 512], BF16, tag="pt")
                            nc.scalar.activation(
                                out=pt[:, ds(w0, 512 - w0)],
                                in_=sp[:, ds(w0, 512 - w0)],
                                func=mybir.ActivationFunctionType.Exp,
                                scale=0.125)
                            if r >= 0:   # diagonal: ragged triangle mask
                                nc.vector.tensor_mul(
                                    out=pt[:, ds(r, 128)], in0=pt[:, ds(r, 128)],
                                    in1=tri_t[:])
                            nc.tensor.matmul(
                                pvp[0:HD + 1, ds(w0, 512 - w0)],
                                (v_t[:, i, ds(VW * h, HD + 1)]),
                                (pt[:, ds(w0, 512 - w0)]),
                                start=(i == 0), stop=(i == nk - 1))
                        # denominator: reciprocal row, broadcast into the
                        # unused partitions 64..127 of the same psum bank
                        rec = rrp.tile([1, 512], BF16, tag="rec")
                        nc.vector.reciprocal(out=rec[:], in_=pvp[HD:HD + 1, :])
                        bc = pss.tile([HD, 512], F32, tag='sp')
                        nc.tensor.matmul(bc[:], (ones_t[:]),
                                         (rec[:]), start=True, stop=True)
                        bcs = rrp.tile([HD, 512], BF16, tag="bcs")
                        nc.vector.tensor_copy(out=bcs[:], in_=bc[:])
                        nc.vector.tensor_mul(
                            out=att_t[hp:hp + HD, htl, ds(q0, 512)],
                            in0=pvp[0:HD, :],
                            in1=bcs[:])

            if dbg:
                nc.sync.dma_start(out=kt_out[:],
                                  in_=kt[:].rearrange("p t s -> p (t s)"))
                nc.sync.dma_start(out=vt_out[:],
                                  in_=v_t[:].rearrange("p a b -> p (a b)"))
                nc.sync.dma_start(out=att_out[:],
                                  in_=att_t[:].rearrange("p t s -> p (t s)"))

            # =========== AllToAll: head-sharded -> seq-sharded ===========
            a2a_in = dram.tile([NCORES, DPC, SSL], BF16)
            a2a_out = dram.tile([S, SSL], BF16)
            for t in range(2):
                nc.sync.dma_start(
                    out=a2a_in[:, ts(t, 128), :].rearrange("d p s -> p d s"),
                    in_=att_t[:, t, :].rearrange("p (d s) -> p d s", d=NCORES))
            if collective:
                nc.gpsimd.collective_compute(
                    "AllToAll",
                    mybir.AluOpType.bypass,
                    replica_groups=[list(range(NCORES))],
                    ins=[a2a_in[:].opt()],
                    outs=[a2a_out[:].opt()],
                )
            else:
                # timeline-sim mock: same-size DRAM->DRAM move
                nc.sync.dma_start(
                    out=a2a_out[:],
                    in_=a2a_in[:].rearrange("d p s -> (d p) s"))

            # =========== o-projection (sequence-parallel) ===========
            with (
                tc.tile_pool(name="af", bufs=1) as afp,
                tc.tile_pool(name="ob", bufs=1) as obp,
                tc.tile_pool(name="pso", bufs=8, space="PSUM") as pso,
            ):
                afull = afp.tile([128, 16, SSL], BF16)
                nc.sync.dma_start(
                    out=afull[:],
                    in_=a2a_out[:].rearrange("(n p) s -> p n s", p=128))
                if dbg:
                    nc.sync.dma_start(out=afu_out[:],
                                      in_=afull[:].rearrange("p a b -> p (a b)"))
                osb = obp.tile([128, 2, HID], F32)
                po = [[pso.tile([128, 512], F32, tag='po', name='po')
                       for t in range(2)] for ob in range(4)]
                for hc in range(16):
                    for ob in range(4):
                        for t in range(2):
                            nc.tensor.matmul(
                                po[ob][t][:],
                                (afull[:, hc, ts(t, 128)]),
                                (ow_t[:, hc, ts(ob, 512)]),
                                start=(hc == 0), stop=(hc == 15))
                for t in range(2):
                    for ob in range(4):
                        nc.scalar.copy(out=osb[:, t, ts(ob, 512)],
                                       in_=po[ob][t][:])
                    nc.sync.dma_start(out=out_sl[ts(t, 128), :],
                                      in_=osb[:, t, :])

    nc.compile()
    return nc


_PROGRAM = None


def _host_inputs(hidden_states, qk_w, v_w, o_w, position_ids):
    bf16 = ml_dtypes.bfloat16
    hs = np.asarray(hidden_states, dtype=np.float32)[0]          # [S, HID]
    qk_w = np.asarray(qk_w, dtype=np.float32)
    v_w = np.asarray(v_w, dtype=np.float32)
    o_w = np.asarray(o_w, dtype=np.float32)
    pos = np.asarray(position_ids)[0].astype(np.float64)         # [S]

    hidT = np.ascontiguousarray(hs.T)                            # [HID, S]
    hidT_bf = hidT.astype(bf16)
    owT_bf = np.ascontiguousarray(o_w.T).astype(bf16)            # [HID, HID]

    inv_freq = 1.0 / (ROPE_THETA ** (np.arange(0, HD, 2, dtype=np.float64) / HD))
    freqs = pos[None, :] * inv_freq[:, None]                     # [32, S]
    emb = np.concatenate([freqs, freqs], axis=0)                 # [64, S]
    cos1 = np.cos(emb).astype(np.float32)
    sin1 = np.sin(emb).astype(np.float32)
    sin_signed = sin1.copy()
    sin_signed[:HD // 2] *= -1.0                                 # fold rotate sign
    cosT = np.tile(cos1, (2, 1)).astype(bf16)                    # [128, S]
    sinT = np.tile(sin_signed, (2, 1)).astype(bf16)

    kl = np.arange(128)[:, None]
    u = np.arange(128)[None, :]
    triT = (u >= kl).astype(bf16)                                # [128, 128]

    # rotate-half row permutation (symmetric involution, per 64-row head)
    idx = np.arange(128)
    src = (idx // HD) * HD + (idx % HD + HD // 2) % HD
    permT = np.zeros((128, 128), np.float32)
    permT[idx, src] = 1.0
    permT = permT.astype(bf16)

    in_maps = []
    for c in range(NCORES):
        rows = slice(DPC * c, DPC * (c + 1))
        xT = hidT[rows]                                          # [256, S] fp32
        # host-side X RoPE: x*cos + rotate_half(x)*sin per 64-row head
        xTs = np.empty_like(xT)
        for h in range(HPC):
            b = HD * h
            xTs[b:b + 32] = -xT[b + 32:b + HD]
            xTs[b + 32:b + HD] = xT[b:b + 32]
        cs = np.tile(cos1, (HPC, 1))                             # [256, S]
        sn = np.tile(sin1, (HPC, 1))
        xTr = (xT * cs + xTs * sn).astype(bf16)
        parts = {
            "hidT": hidT_bf,
            "qkwT": np.ascontiguousarray(qk_w[rows].T).astype(bf16),
            "vwT": np.ascontiguousarray(v_w[rows].T).astype(bf16),
            "owT": owT_bf,
            "xT": xTr,
            "cosT": cosT,
            "sinT": sinT,
            "triT": triT,
            "permT": permT,
        }
        blob = np.concatenate([parts[nm].ravel() for nm, _ in _BLOB_SPEC])
        assert blob.size == BLOB_ELEMS
        in_maps.append({"blob": blob})
    return in_maps


def kernel(hidden_states, qk_w, v_w, o_w, position_ids, **extra):
    global _PROGRAM
    if _PROGRAM is None:
        _PROGRAM = build_program()
    in_maps = _host_inputs(hidden_states, qk_w, v_w, o_w, position_ids)
    res = run_bass_kernel_spmd(_PROGRAM, in_maps, list(range(NCORES)))
    out = np.concatenate([res.results[c]["out_slice"]
                          for c in range(NCORES)], axis=0)
    return out.reshape(1, S, HID).astype(np.float32)


# revision 34
# speedup vs baseline: 1.3208x; 1.1670x over previous
"""InternLM3 custom attention on 8 TRN2 NeuronCores.

Sharding: heads 4-per-core for K/V projection + attention (qk_w/v_w
column-parallel by head); AllToAll converts the attention output from
head-sharded to sequence-sharded; o-projection runs sequence-parallel
(full o_w per core) so each core emits a [256, 2048] output slice.

v2: bf16 streaming path (fp32 PSUM accumulation), software-pipelined
projection -> RoPE -> attention per 512-sequence chunk so the exp
(Activation engine) overlaps the projection matmuls (PE), big resident
SBUF tiles loaded with few large DMAs split across both HWDGE queues
(SP + Activation), o_w prefetched during attention. X-RoPE is folded
into host prep. Attention is computed transposed (S^T[k, q]) so softmax
probabilities feed the PV matmul directly; the softmax denominator
rides along as a ones column appended to V, and its broadcast
reciprocal shares the attention PSUM bank (partitions 64..127).
Causality: strictly-upper k-blocks skipped; diagonal blocks compute
exp/PV only on columns >= r with one [128,128] triangular mask.
"""

import sys

sys.path.insert(0, "/opt/trn_rl_repo")

import numpy as np
import ml_dtypes

import concourse.bass as bass
import concourse.tile as tile
from concourse import bacc, mybir
from concourse.bass import ds, ts
from concourse.bass_utils import run_bass_kernel_spmd

F32 = mybir.dt.float32
BF16 = mybir.dt.bfloat16
NCORES = 8
S = 2048          # sequence
HID = 2048        # hidden
NH = 32           # total heads
HD = 64           # head dim
HPC = NH // NCORES      # heads per core = 4
DPC = HPC * HD          # head-dims per core = 256
SSL = S // NCORES       # output seq slice per core = 256
VW = 66                 # interleaved V stride: 64 dims + 1 ones + 1 pad
ROPE_THETA = 10000.0

# packed input blob layout (bf16 elements, per core)
_BLOB_SPEC = [
    ("hidT", HID * S),
    ("qkwT", HID * DPC),
    ("vwT", HID * DPC),
    ("owT", HID * HID),
    ("xT", DPC * S),
    ("cosT", 128 * S),
    ("sinT", 128 * S),
    ("triT", 128 * 128),
    ("permT", 128 * 128),
]
BLOB_OFFS = {}
_off = 0
for _nm, _n in _BLOB_SPEC:
    BLOB_OFFS[_nm] = (_off, _n)
    _off += _n
BLOB_ELEMS = _off


def build_program(collective=True, dbg=False, split_a2a=False):
    nc = bacc.Bacc("TRN2", target_bir_lowering=False, debug=False,
                   num_devices=NCORES)

    # ---- I/O: one packed bf16 input blob (fewer per-iteration buffer
    # ---- handles on the dispatch path); fp32 out ----
    blob = nc.dram_tensor("blob", [BLOB_ELEMS], BF16, kind="ExternalInput").ap()

    def bslice(name):
        off, n = BLOB_OFFS[name]
        return blob[ds(off, n)]

    hidT = bslice("hidT").rearrange("(n p s) -> p n s", p=128, s=S)
    qkwT = bslice("qkwT").rearrange("(n p d) -> p n d", p=128, d=DPC)
    vwT = bslice("vwT").rearrange("(n p d) -> p n d", p=128, d=DPC)
    owT = bslice("owT").rearrange("(n p d) -> p n d", p=128, d=HID)
    xT_in = bslice("xT").rearrange("(t p s) -> p t s", p=128, s=S)
    cosT = bslice("cosT").rearrange("(p s) -> p s", p=128)
    sinT = bslice("sinT").rearrange("(p s) -> p s", p=128)
    triT = bslice("triT").rearrange("(p q) -> p q", p=128)
    permT = bslice("permT").rearrange("(p q) -> p q", p=128)
    out_sl = nc.dram_tensor("out_slice", [SSL, HID], F32,
                            kind="ExternalOutput").ap()
    if dbg:
        kt_out = nc.dram_tensor("kt_out", [128, 2 * S], BF16,
                                kind="ExternalOutput").ap()
        vt_out = nc.dram_tensor("vt_out", [128, 16 * VW * HPC], BF16,
                                kind="ExternalOutput").ap()
        att_out = nc.dram_tensor("att_out", [128, 2 * S], BF16,
                                 kind="ExternalOutput").ap()
        afu_out = nc.dram_tensor("afu_out", [128, 16 * SSL], BF16,
                                 kind="ExternalOutput").ap()

    with tile.TileContext(nc) as tc:
        with (
            nc.allow_low_precision(reason="bf16 streaming, fp32 psum accum"),
            tc.tile_pool(name="const", bufs=1) as const,
            tc.tile_pool(name="dram", bufs=1, space="DRAM") as dram,
        ):
            # ---- persistent SBUF residents ----
            qkw_t = const.tile([128, 16, DPC], BF16)
            vw_t = const.tile([128, 16, DPC], BF16)
            xt = const.tile([128, 2, S], BF16)      # X^T, rope'd on host
            cos_t = const.tile([128, S], BF16)
            sin_t = const.tile([128, S], BF16)
            tri_t = const.tile([128, 128], BF16)
            perm_t = const.tile([128, 128], BF16)
            ow_t = const.tile([128, 16, HID], BF16)
            kt = const.tile([128, 2, S], BF16)      # K^T, rope'd in place
            v_t = const.tile([128, 16, VW * HPC], BF16)
            att_t = const.tile([128, 2, S], BF16)   # attn^T assembled
            ones_t = const.tile([1, HD], BF16)

            nc.vector.memset(ones_t[:], 1.0)
            # ones column of V (denominator accumulator)
            nc.vector.memset(
                v_t[:].rearrange("p st (h w) -> p st h w", w=VW)[:, :, :, HD:HD + 1],
                1.0)

            # ===== pipelined: per 512-seq chunk, project K/V, rope K,
            # ===== then attention q-block j=sq (needs K/V chunks <= sq).
            with (
                tc.tile_pool(name="hidp", bufs=1) as hidp,
                tc.tile_pool(name="psk", bufs=2, space="PSUM") as psk,
                tc.tile_pool(name="psv", bufs=2, space="PSUM") as psv,
                tc.tile_pool(name="pss", bufs=2, space="PSUM") as pss,
                tc.tile_pool(name="pspv", bufs=2, space="PSUM") as pspv,
                tc.tile_pool(name="sw", bufs=2) as swp,
                tc.tile_pool(name="pp", bufs=4) as ppool,
                tc.tile_pool(name="rr", bufs=2) as rrp,
            ):
                hid_t = hidp.tile([128, 16, S], BF16)
                # SP queue: split first loads so K matmuls start early;
                # vw only needed once the K half of chunk 0 is done.
                nc.sync.dma_start(out=qkw_t[:, 0:2, :], in_=qkwT[:, 0:2, :])
                nc.sync.dma_start(out=hid_t[:, 0:2, ts(0, 512)],
                                  in_=hidT[:, 0:2, ts(0, 512)])
                nc.sync.dma_start(out=qkw_t[:, 2:8, :], in_=qkwT[:, 2:8, :])
                nc.sync.dma_start(out=hid_t[:, 2:8, ts(0, 512)],
                                  in_=hidT[:, 2:8, ts(0, 512)])
                nc.sync.dma_start(out=qkw_t[:, 8:16, :], in_=qkwT[:, 8:16, :])
                nc.sync.dma_start(out=hid_t[:, 8:16, ts(0, 512)],
                                  in_=hidT[:, 8:16, ts(0, 512)])
                nc.sync.dma_start(out=vw_t[:], in_=vwT)
                for sq in range(1, 4):
                    nc.sync.dma_start(out=hid_t[:, :, ts(sq, 512)],
                                      in_=hidT[:, :, ts(sq, 512)])
                # o_w prefetch: after the phase A loads so it doesn't
                # steal DMA bandwidth from them; lands well before o-proj.
                nc.sync.dma_start(out=ow_t[:], in_=owT)
                # ACT queue: rope/attention consts (small, needed early).
                nc.scalar.dma_start(out=cos_t[:], in_=cosT)
                nc.scalar.dma_start(out=sin_t[:], in_=sinT)
                nc.scalar.dma_start(out=xt[:], in_=xT_in)
                nc.scalar.dma_start(out=tri_t[:], in_=triT)
                nc.scalar.dma_start(out=perm_t[:], in_=permT)
                for sq in range(4):
                    sqs = ds(512 * sq, 512)
                    # ---- phase A chunk: K^T then V for seq block sq ----
                    pk = [psk.tile([128, 512], F32, tag='pk', name='pk')
                          for _ in range(2)]
                    for hc in range(16):
                        for m in range(2):
                            nc.tensor.matmul(
                                pk[m][:],
                                (qkw_t[:, hc, ts(m, 128)]),
                                (hid_t[:, hc, sqs]),
                                start=(hc == 0), stop=(hc == 15))
                    # K: copy to bf16, rotate-half via PE permutation matmul,
                    # rope in place (sin sign folded on host)
                    ks = swp.tile([128, 2, 512], BF16, tag="sw")
                    for t in range(2):
                        nc.vector.tensor_copy(out=kt[:, t, sqs], in_=pk[t][:])
                    for t in range(2):
                        ksp = pss.tile([128, 512], F32, tag='sp')
                        nc.tensor.matmul(ksp[:], (perm_t[:]),
                                         (kt[:, t, sqs]),
                                         start=True, stop=True)
                        nc.vector.tensor_mul(out=ks[:, t, :], in0=ksp[:],
                                             in1=sin_t[:, sqs])
                        nc.vector.tensor_mul(out=kt[:, t, sqs],
                                             in0=kt[:, t, sqs],
                                             in1=cos_t[:, sqs])
                        nc.vector.tensor_add(out=kt[:, t, sqs],
                                             in0=kt[:, t, sqs],
                                             in1=ks[:, t, :])
                    # V: one psum group per bank slot (start=True zeroes the
                    # whole 2KB zero-region, so groups must not share a bank)
                    for st4 in range(4):
                        pvt = psv.tile([128, 256], F32, tag='pv', name='pv')
                        for hc in range(16):
                            nc.tensor.matmul(
                                pvt[:],
                                (hid_t[:, hc, ds(512 * sq + 128 * st4, 128)]),
                                (vw_t[:, hc, :]),
                                start=(hc == 0), stop=(hc == 15))
                        nc.vector.tensor_copy(
                            out=v_t[:, sq * 4 + st4, :].rearrange(
                                "p (h w) -> p h w", w=VW)[:, :, 0:HD],
                            in_=pvt[:].rearrange("p (h d) -> p h d", d=HD))

                    # ---- phase B: attention q-block j == sq, all 4 heads ----
                    j = sq
                    q0 = 512 * j
                    nk = 4 * (j + 1)
                    for h in range(HPC):
                        hp = HD * (h % 2)
                        htl = h // 2
                        pvp = pspv.tile([HD + 1, 512], F32, tag='pvp')
                        for i in range(nk):
                            r = 128 * i - q0
                            w0 = max(r, 0)
                            sp = pss.tile([128, 512], F32, tag='sp')
                            nc.tensor.matmul(
                                sp[:, ds(w0, 512 - w0)],
                                (kt[hp:hp + HD, htl, ts(i, 128)]),
                                (xt[hp:hp + HD, htl, ds(q0 + w0, 512 - w0)]),
                                start=True, stop=True)
                            pt = ppool.tile([128, 512], BF16, tag="pt")
                            nc.scalar.activation(
                                out=pt[:, ds(w0, 512 - w0)],
                                in_=sp[:, ds(w0, 512 - w0)],
                                func=mybir.ActivationFunctionType.Exp,
                                scale=0.125)
                            if r >= 0:   # diagonal: ragged triangle mask
                                nc.vector.tensor_mul(
                                    out=pt[:, ds(r, 128)], in0=pt[:, ds(r, 128)],
                                    in1=tri_t[:])
                            nc.tensor.matmul(
                                pvp[0:HD + 1, ds(w0, 512 - w0)],
                                (v_t[:, i, ds(VW * h, HD + 1)]),
                                (pt[:, ds(w0, 512 - w0)]),
                                start=(i == 0), stop=(i == nk - 1))
                        # denominator: reciprocal row, broadcast into the
                        # unused partitions 64..127 of the same psum bank
                        rec = rrp.tile([1, 512], BF16, tag="rec")
                        nc.vector.reciprocal(out=rec[:], in_=pvp[HD:HD + 1, :])
                        bc = pss.tile([HD, 512], F32, tag='sp')
                        nc.tensor.matmul(bc[:], (ones_t[:]),
                                         (rec[:]), start=True, stop=True)
                        bcs = rrp.tile([HD, 512], BF16, tag="bcs")
                        nc.vector.tensor_copy(out=bcs[:], in_=bc[:])
                        nc.vector.tensor_mul(
                            out=att_t[hp:hp + HD, htl, ds(q0, 512)],
                            in0=pvp[0:HD, :],
                            in1=bcs[:])

            if dbg:
                nc.sync.dma_start(out=kt_out[:],
                                  in_=kt[:].rearrange("p t s -> p (t s)"))
                nc.sync.dma_start(out=vt_out[:],
                                  in_=v_t[:].rearrange("p a b -> p (a b)"))
                nc.sync.dma_start(out=att_out[:],
                                  in_=att_t[:].rearrange("p t s -> p (t s)"))

            # =========== AllToAll: head-sharded -> seq-sharded ===========
            # Split by head-pair tile t: the t=0 collective fires as soon as
            # heads 0-1 finish, overlapping the t=1 attention tail and the
            # even-hc half of the o-projection with the t=1 collective.
            a2a_in = [dram.tile([NCORES, 128, SSL], BF16, name=f"a2ain{t}")
                      for t in range(2)]
            a2a_out = [dram.tile([NCORES * 128, SSL], BF16, name=f"a2aout{t}")
                       for t in range(2)]
            for t in range(2):
                nc.sync.dma_start(
                    out=a2a_in[t][:].rearrange("d p s -> p d s"),
                    in_=att_t[:, t, :].rearrange("p (d s) -> p d s", d=NCORES))
                if collective:
                    nc.gpsimd.collective_compute(
                        "AllToAll",
                        mybir.AluOpType.bypass,
                        replica_groups=[list(range(NCORES))],
                        ins=[a2a_in[t][:].opt()],
                        outs=[a2a_out[t][:].opt()],
                    )
                else:
                    # timeline-sim mock: same-size DRAM->DRAM move
                    nc.sync.dma_start(
                        out=a2a_out[t][:],
                        in_=a2a_in[t][:].rearrange("d p s -> (d p) s"))

            # =========== o-projection (sequence-parallel) ===========
            with (
                tc.tile_pool(name="af", bufs=1) as afp,
                tc.tile_pool(name="ob", bufs=1) as obp,
                tc.tile_pool(name="pso", bufs=8, space="PSUM") as pso,
            ):
                afull = afp.tile([128, 16, SSL], BF16)
                for t in range(2):
                    # a2a_out[t] rows (d p) hold global attn dims 256d+128t+p
                    # -> afull n slices t, t+2, t+4, ...
                    nc.sync.dma_start(
                        out=afull[:].rearrange(
                            "p (d u) s -> p d u s", u=2)[:, :, t, :],
                        in_=a2a_out[t][:].rearrange("(d p) s -> p d s", p=128))
                if dbg:
                    nc.sync.dma_start(out=afu_out[:],
                                      in_=afull[:].rearrange("p a b -> p (a b)"))
                osb = obp.tile([128, 2, HID], F32)
                po = [[pso.tile([128, 512], F32, tag='po', name='po')
                       for t in range(2)] for ob in range(4)]
                # even hc chunks depend only on the t=0 collective, odd on
                # t=1: run all even ones first so they overlap collective 1.
                for hc in [2 * i for i in range(8)] + [2 * i + 1 for i in range(8)]:
                    for ob in range(4):
                        for t in range(2):
                            nc.tensor.matmul(
                                po[ob][t][:],
                                (afull[:, hc, ts(t, 128)]),
                                (ow_t[:, hc, ts(ob, 512)]),
                                start=(hc == 0), stop=(hc == 15))
                for ob in range(4):
                    nc.scalar.copy(out=osb[:, 0, ts(ob, 512)],
                                   in_=po[ob][0][:])
                    nc.vector.tensor_copy(out=osb[:, 1, ts(ob, 512)],
                                          in_=po[ob][1][:])
                nc.sync.dma_start(out=out_sl[ts(0, 128), :], in_=osb[:, 0, :])
                nc.scalar.dma_start(out=out_sl[ts(1, 128), :], in_=osb[:, 1, :])

    nc.compile()
    return nc


_PROGRAM = None


def _host_inputs(hidden_states, qk_w, v_w, o_w, position_ids):
    bf16 = ml_dtypes.bfloat16
    hs = np.asarray(hidden_states, dtype=np.float32)[0]          # [S, HID]
    qk_w = np.asarray(qk_w, dtype=np.float32)
    v_w = np.asarray(v_w, dtype=np.float32)
    o_w = np.asarray(o_w, dtype=np.float32)
    pos = np.asarray(position_ids)[0].astype(np.float64)         # [S]

    hidT = np.ascontiguousarray(hs.T)                            # [HID, S]
    hidT_bf = hidT.astype(bf16)
    owT_bf = np.ascontiguousarray(o_w.T).astype(bf16)            # [HID, HID]

    inv_freq = 1.0 / (ROPE_THETA ** (np.arange(0, HD, 2, dtype=np.float64) / HD))
    freqs = pos[None, :] * inv_freq[:, None]                     # [32, S]
    emb = np.concatenate([freqs, freqs], axis=0)                 # [64, S]
    cos1 = np.cos(emb).astype(np.float32)
    sin1 = np.sin(emb).astype(np.float32)
    sin_signed = sin1.copy()
    sin_signed[:HD // 2] *= -1.0                                 # fold rotate sign
    cosT = np.tile(cos1, (2, 1)).astype(bf16)                    # [128, S]
    sinT = np.tile(sin_signed, (2, 1)).astype(bf16)

    kl = np.arange(128)[:, None]
    u = np.arange(128)[None, :]
    triT = (u >= kl).astype(bf16)                                # [128, 128]

    # rotate-half row permutation (symmetric involution, per 64-row head)
    idx = np.arange(128)
    src = (idx // HD) * HD + (idx % HD + HD // 2) % HD
    permT = np.zeros((128, 128), np.float32)
    permT[idx, src] = 1.0
    permT = permT.astype(bf16)

    in_maps = []
    for c in range(NCORES):
        rows = slice(DPC * c, DPC * (c + 1))
        xT = hidT[rows]                                          # [256, S] fp32
        # host-side X RoPE: x*cos + rotate_half(x)*sin per 64-row head
        xTs = np.empty_like(xT)
        for h in range(HPC):
            b = HD * h
            xTs[b:b + 32] = -xT[b + 32:b + HD]
            xTs[b + 32:b + HD] = xT[b:b + 32]
        cs = np.tile(cos1, (HPC, 1))                             # [256, S]
        sn = np.tile(sin1, (HPC, 1))
        xTr = (xT * cs + xTs * sn).astype(bf16)
        parts = {
            "hidT": hidT_bf,
            "qkwT": np.ascontiguousarray(qk_w[rows].T).astype(bf16),
            "vwT": np.ascontiguousarray(v_w[rows].T).astype(bf16),
            "owT": owT_bf,
            "xT": xTr,
            "cosT": cosT,
            "sinT": sinT,
            "triT": triT,
            "permT": permT,
        }
        blob = np.concatenate([parts[nm].ravel() for nm, _ in _BLOB_SPEC])
        assert blob.size == BLOB_ELEMS
        in_maps.append({"blob": blob})
    return in_maps


def kernel(hidden_states, qk_w, v_w, o_w, position_ids, **extra):
    global _PROGRAM
    if _PROGRAM is None:
        _PROGRAM = build_program()
    in_maps = _host_inputs(hidden_states, qk_w, v_w, o_w, position_ids)
    res = run_bass_kernel_spmd(_PROGRAM, in_maps, list(range(NCORES)))
    out = np.concatenate([res.results[c]["out_slice"]
                          for c in range(NCORES)], axis=0)
    return out.reshape(1, S, HID).astype(np.float32)
